# revision 17
# baseline (speedup 1.0000x reference)
import os
import sys

import numpy as np

for _p in ("/root/.axon_site", "/root/.axon_site/_ro/trn_rl_repo",
           "/root/.axon_site/_ro/pypackages"):
    if os.path.isdir(_p) and _p not in sys.path:
        sys.path.append(_p)

import ml_dtypes

N, C, H, W = 4, 19, 384, 384
K = 3
HP = WP = H - K + 1
N_CORES = 8
ROWS_PER_CORE = 192
SHIFTS = [(0, 0), (0, 1), (0, 2)] + [(dr, dc) for dr in (1, 2) for dc in (-2, -1, 0, 1, 2)]
NS = len(SHIFTS)
NONCENTER = [si for si in range(NS) if SHIFTS[si] != (0, 0)]
LGROWS = 196
P1ROWS = 68
DW0, DW1 = W, 194
GROUP_X0 = [0, 190]
COLS = 2 * NS
BCOLS = 2 * NS * 4

ORDER0 = [(0, 0), (0, 2), (0, 1), (1, 0), (1, -1), (1, 1), (1, 2), (1, -2),
          (2, 0), (2, 1), (2, 2), (2, -1), (2, -2)]
ORDER1 = list(ORDER0)

CONVERT0 = {(1, -1), (1, 1), (1, 2), (1, -2), (2, -1), (2, 1), (2, 2), (2, -2)}
CONVERT1 = {(1, -1), (1, 1), (1, 2), (1, -2)}
RT_HALF = 0.7071067811865476


def _wx_profile(dc, x):
    w = np.zeros_like(x, dtype=np.float64)
    for ca in range(K):
        if 0 <= ca + dc < K:
            w += ((x - ca >= 0) & (x - ca < WP))
    return w


def _wy_profile(dr, y):
    w = np.zeros_like(y, dtype=np.float64)
    for ra in range(K):
        if 0 <= ra + dr < K:
            w += ((y - ra >= 0) & (y - ra < HP))
    return w


def _border_weights():
    bw = np.zeros((128, 2 * NS * 4), np.float64)
    for pi, DW in enumerate((DW0, DW1)):
        for si, (dr, dc) in enumerate(SHIFTS):
            wxc = sum(1 for ca in range(K) if 0 <= ca + dc < K)
            for p in range(128):
                if pi == 0:
                    gx0, own_lo, own_hi = 0, 0, W
                else:
                    g = p // 64
                    gx0 = GROUP_X0[g]
                    own_lo, own_hi = (0, 192) if g == 0 else (192, W)
                for bi, j in enumerate((0, 1, DW - 2, DW - 1)):
                    x = gx0 + j
                    if own_lo <= x < own_hi and 0 <= x + dc < W and x < W:
                        wx = _wx_profile(dc, np.array([x]))[0]
                    else:
                        wx = 0.0
                    bw[p, (pi * NS + si) * 4 + bi] = wx - wxc
    return bw


_BW = None


def _sign_maps(lbp):
    shm0 = np.zeros((128, 12, DW0), np.float32)
    shm1 = np.zeros((128, 12, DW1), np.float32)
    rows0 = np.arange(128)
    for j, si in enumerate(NONCENTER):
        dr, dc = SHIFTS[si]
        a = lbp[rows0]
        b = lbp[rows0 + dr]
        eq = np.zeros((128, W), bool)
        lo, hi = max(0, -dc), min(W, W - dc)
        eq[:, lo:hi] = a[:, lo:hi] == b[:, lo + dc:hi + dc]
        shm0[:, j, :] = np.where(eq, -1.0, 1.0)
        for g in range(2):
            x0 = GROUP_X0[g]
            rows = 128 + np.arange(64)
            a1 = lbp[rows][:, x0:x0 + DW1]
            eq1 = np.zeros((64, DW1), bool)
            xs = np.arange(x0, x0 + DW1) + dc
            ok = (xs >= 0) & (xs < W)
            eq1[:, ok] = a1[:, ok] == lbp[rows + dr][:, xs[ok]]
            shm1[64 * g:64 * g + 64, j, :] = np.where(eq1, -1.0, 1.0)
    return shm0.reshape(128, 12 * DW0), shm1.reshape(128, 12 * DW1)


def _host_inputs(logits, labels):
    in_maps = []
    for k in range(N_CORES):
        img, half = k // 2, k % 2
        g0 = half * ROWS_PER_CORE
        hi = min(H, g0 + LGROWS)
        lg = np.zeros((C, LGROWS, W), np.float32)
        lg[:, : hi - g0] = logits[img, :, g0:hi]
        lbp = np.full((LGROWS, W), -1.0, np.float32)
        lbp[: hi - g0] = labels[img, g0:hi].astype(np.float32)

        lgf = np.zeros((132, 2 + C * DW0 + 2), np.float32)
        lgf[:131, 2:2 + C * DW0] = (
            lg[:, 0:131].transpose(1, 0, 2).reshape(131, C * DW0))
        lgp1 = np.zeros((2, P1ROWS, 2 + C * DW1 + 2), np.float32)
        for g in range(2):
            x0 = GROUP_X0[g]
            lgp1[g, :, 2:2 + C * DW1] = (
                lg[:, 128:128 + P1ROWS, x0:x0 + DW1]
                .transpose(1, 0, 2).reshape(P1ROWS, C * DW1))
        shm0, shm1 = _sign_maps(lbp)
        in_maps.append({
            "lgf": lgf.astype(ml_dtypes.bfloat16),
            "lgp1": lgp1.astype(ml_dtypes.bfloat16),
            "shm0": shm0.astype(ml_dtypes.bfloat16),
            "shm1": shm1.astype(ml_dtypes.bfloat16),
            "ident": np.eye(128, dtype=np.float32).astype(ml_dtypes.bfloat16),
        })
    return in_maps


def _combine(accs_list, bcols_list):
    global _BW
    if _BW is None:
        _BW = _border_weights()
    total = 0.0
    for k in range(N_CORES):
        acc = accs_list[k].astype(np.float64)
        bc = bcols_list[k].astype(np.float64)
        g0 = (k % 2) * ROWS_PER_CORE
        for pi in range(2):
            p = np.arange(128)
            gy = g0 + p if pi == 0 else g0 + 128 + (p % 64)
            for si, (dr, dc) in enumerate(SHIFTS):
                mult = 1.0 if (dr, dc) == (0, 0) else 2.0
                wxc = float(sum(1 for ca in range(K) if 0 <= ca + dc < K))
                wy = _wy_profile(dr, gy)
                idx = pi * NS + si
                wb = _BW[:, idx * 4: idx * 4 + 4]
                full = acc[:, idx]
                border = (bc[:, idx * 4: idx * 4 + 4] * wb).sum(1)
                total += mult * np.sum(wy * (wxc * full + border))
    return total / (N * 81 * HP * WP)


_NC = None


def _build():
    global _NC
    if _NC is not None:
        return _NC
    from concourse import bacc, mybir
    import concourse.tile as tile

    f32 = mybir.dt.float32
    bf16 = mybir.dt.bfloat16
    Alu = mybir.AluOpType
    AF = mybir.ActivationFunctionType

    from concourse.hw_specs import get_activation_tables as _gat
    _keep = "natural_log_exp_and_others"
    _mine = {AF.Exp, AF.Ln, AF.Square, AF.Copy}

    def _gat_filtered(arch):
        t = _gat(arch)
        for name in t:
            if name != _keep:
                t[name] = t[name] - _mine
        return t

    bacc.get_activation_tables = _gat_filtered

    nc = bacc.Bacc("TRN2", target_bir_lowering=False, debug=False, num_devices=N_CORES)
    lgf = nc.dram_tensor("lgf", (132, 2 + C * DW0 + 2), bf16, kind="ExternalInput")
    lgp1 = nc.dram_tensor("lgp1", (2, P1ROWS, 2 + C * DW1 + 2), bf16,
                          kind="ExternalInput")
    shm0 = nc.dram_tensor("shm0", (128, 12 * DW0), bf16, kind="ExternalInput")
    shm1 = nc.dram_tensor("shm1", (128, 12 * DW1), bf16, kind="ExternalInput")
    identd = nc.dram_tensor("ident", (128, 128), bf16, kind="ExternalInput")
    accs = nc.dram_tensor("accs", (128, COLS), f32, kind="ExternalOutput")
    bcols = nc.dram_tensor("bcols", (128, BCOLS), bf16, kind="ExternalOutput")

    with tile.TileContext(nc) as tc:
        with tc.tile_pool(name="persist", bufs=1) as pool, \
             tc.tile_pool(name="work", bufs=2) as wpool, \
             tc.tile_pool(name="psum", bufs=2, space="PSUM") as psum_pool:
            accs_t = pool.tile([128, COLS], f32, name="accs_t")
            idt = pool.tile([128, 128], bf16, name="idt")
            nc.sync.dma_start(idt[:, :], identd[:, :])

            for pi, (DW, order) in enumerate(((DW0, ORDER0), (DW1, ORDER1))):
                FW = C * DW
                T = {}
                for dr in range(K):
                    t = pool.tile([128, FW + 4], bf16, tag=f"T{dr}_{pi}",
                                  name=f"T{dr}_{pi}")
                    if pi == 0:
                        nchunk = 16 if dr == 0 else 8
                        bnds = [round((FW + 4) * i / nchunk) for i in range(nchunk + 1)]
                        for ci, (c0, c1) in enumerate(zip(bnds[:-1], bnds[1:])):
                            eng = nc.gpsimd if ci % 2 == 0 else nc.sync
                            eng.dma_start(t[:, c0:c1], lgf[dr:dr + 128, c0:c1])
                    else:
                        for g in range(2):
                            nc.gpsimd.dma_start(t[64 * g:64 * g + 64, :],
                                                lgp1[g, dr:dr + 64, :])
                    T[dr] = t

                shm_t = pool.tile([128, 12 * DW], bf16, tag=f"shm_{pi}",
                                  name=f"shm_{pi}")
                shmd = shm0 if pi == 0 else shm1
                half = 6 * DW
                nc.sync.dma_start(shm_t[:, 0:half], shmd[:, 0:half])
                nc.sync.dma_start(shm_t[:, half:], shmd[:, half:])

                l1 = pool.tile([128, NS, DW], bf16, tag=f"l1_{pi}", name=f"l1_{pi}")

                convert = CONVERT0 if pi == 0 else CONVERT1
                qh = {}
                for d in sorted({0} | {dr for (dr, dc) in convert}):
                    q = pool.tile([128, DW + 4], bf16, tag=f"qh{d}_{pi}",
                                  name=f"qh{d}_{pi}")
                    nc.gpsimd.memset(q[:, 0:2], 0)
                    nc.gpsimd.memset(q[:, DW + 2:DW + 4], 0)
                    qh[d] = q

                def tree(pb, out_ap):
                    nc.vector.tensor_tensor(pb[:, 0:8 * DW], pb[:, 0:8 * DW],
                                            pb[:, 8 * DW:16 * DW], Alu.add)
                    nc.vector.tensor_tensor(pb[:, 0:4 * DW], pb[:, 0:4 * DW],
                                            pb[:, 4 * DW:8 * DW], Alu.add)
                    nc.vector.tensor_tensor(pb[:, 0:3 * DW], pb[:, 0:3 * DW],
                                            pb[:, 16 * DW:19 * DW], Alu.add)
                    nc.vector.tensor_tensor(pb[:, 0:2 * DW], pb[:, 0:2 * DW],
                                            pb[:, 2 * DW:4 * DW], Alu.add)
                    nc.vector.tensor_tensor(out_ap, pb[:, 0:DW],
                                            pb[:, DW:2 * DW], Alu.add)

                def softplus_accum(si, corr_ap, center):
                    idx = pi * NS + si
                    u = wpool.tile([128, DW], f32, tag="u", bufs=4, name=f"u_{pi}_{si}")
                    if center:
                        nc.scalar.activation(u[:, :], corr_ap, AF.Exp, scale=-1.0)
                    else:
                        j = NONCENTER.index(si)
                        wt = wpool.tile([128, DW], bf16, tag="wt", bufs=4,
                                        name=f"wt_{pi}_{si}")
                        nc.vector.tensor_tensor(wt[:, :],
                                                shm_t[:, j * DW:(j + 1) * DW],
                                                corr_ap, Alu.mult)
                        nc.scalar.activation(u[:, :], wt[:, :], AF.Exp)
                    nc.scalar.activation(
                        l1[:, si, :], u[:, :], AF.Ln, bias=1.0,
                        accum_out=accs_t[:, idx:idx + 1])

                def emit_center():
                    si = SHIFTS.index((0, 0))
                    pb = wpool.tile([128, FW], bf16, tag="pb", bufs=4, name=f"pb_{pi}_c")
                    h = (FW // 2) & ~1
                    nc.scalar.activation(pb[:, 0:h], T[0][:, 2:2 + h], AF.Square)
                    nc.scalar.activation(pb[:, h:FW], T[0][:, 2 + h:2 + FW],
                                         AF.Square)
                    corr = wpool.tile([128, DW], bf16, tag="corr", bufs=4,
                                      name=f"corr_{pi}_c")
                    tree(pb, corr[:, :])
                    nc.scalar.activation(qh[0][:, 2:2 + DW], corr[:, :],
                                         AF.Copy, scale=0.5)
                    softplus_accum(si, corr[:, :], True)

                def emit_qaux(d):
                    pb = wpool.tile([128, FW], bf16, tag="pb", bufs=4, name=f"pbq{d}_{pi}")
                    nc.scalar.activation(pb[:, :], T[d][:, 2:2 + FW], AF.Square,
                                         scale=RT_HALF)
                    tree(pb, qh[d][:, 2:2 + DW])

                def emit_q1_assembly():
                    if pi == 0:
                        nc.sync.dma_start(qh[1][0:127, 2:2 + DW],
                                          qh[0][1:128, 2:2 + DW])
                        nc.sync.dma_start(qh[1][127:128, 2:2 + DW],
                                          qh[2][126:127, 2:2 + DW])
                    else:
                        for g in range(2):
                            nc.sync.dma_start(
                                qh[1][64 * g:64 * g + 63, 2:2 + DW],
                                qh[0][64 * g + 1:64 * g + 64, 2:2 + DW])
                            nc.sync.dma_start(
                                qh[1][64 * g + 63:64 * g + 64, 2:2 + DW],
                                qh[2][64 * g + 62:64 * g + 63, 2:2 + DW])

                def emit_slot_dve(dr, dc):
                    si = SHIFTS.index((dr, dc))
                    o1 = 2 + dc
                    pb = wpool.tile([128, FW], bf16, tag="pb", bufs=4,
                                    name=f"pb_{pi}_{si}")
                    corr = wpool.tile([128, DW], bf16, tag="corr", bufs=4,
                                      name=f"corr_{pi}_{si}")
                    nc.vector.tensor_tensor(pb[:, :], T[0][:, 2:2 + FW],
                                            T[dr][:, o1:o1 + FW], Alu.mult)
                    tree(pb, corr[:, :])
                    softplus_accum(si, corr[:, :], False)

                def emit_slot_pe(dr, dc):
                    si = SHIFTS.index((dr, dc))
                    o1 = 2 + dc
                    MMW = 512
                    nch = (FW + MMW - 1) // MMW
                    pb = wpool.tile([128, FW], bf16, tag="pb", bufs=4,
                                    name=f"pbS_{pi}_{si}")
                    done = 0
                    while done < nch:
                        take = min(2, nch - done)
                        ps = psum_pool.tile([128, 1024], f32, tag="ps", bufs=4,
                                            name=f"ps_{pi}_{si}_{done}")
                        for k in range(take):
                            c0 = (done + k) * MMW
                            w = min(MMW, FW - c0)
                            nc.tensor.matmul(ps[:, k * MMW:k * MMW + w],
                                             idt[:, :], T[0][:, 2 + c0:2 + c0 + w],
                                             start=True, stop=False)
                            nc.tensor.matmul(ps[:, k * MMW:k * MMW + w],
                                             idt[:, :],
                                             T[dr][:, o1 + c0:o1 + c0 + w],
                                             start=False, stop=True)
                        w2 = min(1024, FW - done * MMW)
                        nc.scalar.activation(pb[:, done * MMW:done * MMW + w2],
                                             ps[:, 0:w2], AF.Square,
                                             scale=RT_HALF)
                        done += take
                    corr = wpool.tile([128, DW], bf16, tag="corr", bufs=4,
                                      name=f"corr_{pi}_{si}")
                    tree(pb, corr[:, :])
                    nc.vector.tensor_tensor(corr[:, :], corr[:, :],
                                            qh[0][:, 2:2 + DW], Alu.subtract)
                    nc.vector.tensor_tensor(corr[:, :], corr[:, :],
                                            qh[dr][:, o1:o1 + DW], Alu.subtract)
                    softplus_accum(si, corr[:, :], False)

                emit_center()
                for (dr, dc) in order[1:]:
                    if (dr, dc) == (1, 0):
                        emit_slot_dve(1, 0)
                        need = {d for (d2, c2) in convert for d in (d2,)} - {0}
                        if need:
                            if pi == 0:
                                emit_qaux(2)
                            else:
                                emit_qaux(1)
                            if 1 in need and pi == 0:
                                emit_q1_assembly()
                        continue
                    if (dr, dc) in convert:
                        emit_slot_pe(dr, dc)
                    else:
                        emit_slot_dve(dr, dc)

                bc = pool.tile([128, NS, 4], bf16, tag=f"bc_{pi}", name=f"bc_{pi}")
                nc.scalar.copy(bc[:, :, 0:2], l1[:, :, 0:2])
                nc.scalar.copy(bc[:, :, 2:4], l1[:, :, DW - 2:DW])
                nc.sync.dma_start(
                    bcols[:, pi * NS * 4:(pi + 1) * NS * 4],
                    bc[:, :, :].rearrange("p s b -> p (s b)"))
                nc.sync.dma_start(accs[:, pi * NS:(pi + 1) * NS],
                                  accs_t[:, pi * NS:(pi + 1) * NS])

    nc.finalize()
    _NC = nc
    return nc


def kernel(logits, labels):
    nc = _build()
    in_maps = _host_inputs(np.asarray(logits, np.float32), np.asarray(labels))
    from concourse.bass_utils import run_bass_kernel_spmd
    res = run_bass_kernel_spmd(nc, in_maps, core_ids=list(range(N_CORES)))
    accs_list = [res.results[k]["accs"] for k in range(N_CORES)]
    bcols_list = [res.results[k]["bcols"] for k in range(N_CORES)]
    return np.array(_combine(accs_list, bcols_list), np.float32)


# revision 18
# speedup vs baseline: 1.0163x; 1.0163x over previous
import os
import sys

import numpy as np

for _p in ("/root/.axon_site", "/root/.axon_site/_ro/trn_rl_repo",
           "/root/.axon_site/_ro/pypackages"):
    if os.path.isdir(_p) and _p not in sys.path:
        sys.path.append(_p)

import ml_dtypes

N, C, H, W = 4, 19, 384, 384
K = 3
HP = WP = H - K + 1
N_CORES = 8
ROWS_PER_CORE = 192
SHIFTS = [(0, 0), (0, 1), (0, 2)] + [(dr, dc) for dr in (1, 2) for dc in (-2, -1, 0, 1, 2)]
NS = len(SHIFTS)
NONCENTER = [si for si in range(NS) if SHIFTS[si] != (0, 0)]
LGROWS = 196
P1ROWS = 68
DW0, DW1 = W, 194
GROUP_X0 = [0, 190]
COLS = 2 * NS
BCOLS = 2 * NS * 4

ORDER0 = [(0, 0), (0, 2), (0, 1), (1, 0), (1, -1), (1, 1), (1, 2), (1, -2),
          (2, 0), (2, 1), (2, 2), (2, -1), (2, -2)]
ORDER1 = list(ORDER0)

CONVERT0 = {(0, 1), (0, 2), (1, -1), (1, 1), (1, 2), (1, -2)}
CONVERT1 = {(0, 1), (0, 2), (1, -1), (1, 1)}
RT_HALF = 0.7071067811865476


def _wx_profile(dc, x):
    w = np.zeros_like(x, dtype=np.float64)
    for ca in range(K):
        if 0 <= ca + dc < K:
            w += ((x - ca >= 0) & (x - ca < WP))
    return w


def _wy_profile(dr, y):
    w = np.zeros_like(y, dtype=np.float64)
    for ra in range(K):
        if 0 <= ra + dr < K:
            w += ((y - ra >= 0) & (y - ra < HP))
    return w


def _border_weights():
    bw = np.zeros((128, 2 * NS * 4), np.float64)
    for pi, DW in enumerate((DW0, DW1)):
        for si, (dr, dc) in enumerate(SHIFTS):
            wxc = sum(1 for ca in range(K) if 0 <= ca + dc < K)
            for p in range(128):
                if pi == 0:
                    gx0, own_lo, own_hi = 0, 0, W
                else:
                    g = p // 64
                    gx0 = GROUP_X0[g]
                    own_lo, own_hi = (0, 192) if g == 0 else (192, W)
                for bi, j in enumerate((0, 1, DW - 2, DW - 1)):
                    x = gx0 + j
                    if own_lo <= x < own_hi and 0 <= x + dc < W and x < W:
                        wx = _wx_profile(dc, np.array([x]))[0]
                    else:
                        wx = 0.0
                    bw[p, (pi * NS + si) * 4 + bi] = wx - wxc
    return bw


_BW = None


def _sign_maps(lbp):
    shm0 = np.zeros((128, 12, DW0), np.float32)
    shm1 = np.zeros((128, 12, DW1), np.float32)
    rows0 = np.arange(128)
    for j, si in enumerate(NONCENTER):
        dr, dc = SHIFTS[si]
        a = lbp[rows0]
        b = lbp[rows0 + dr]
        eq = np.zeros((128, W), bool)
        lo, hi = max(0, -dc), min(W, W - dc)
        eq[:, lo:hi] = a[:, lo:hi] == b[:, lo + dc:hi + dc]
        shm0[:, j, :] = np.where(eq, -1.0, 1.0)
        for g in range(2):
            x0 = GROUP_X0[g]
            rows = 128 + np.arange(64)
            a1 = lbp[rows][:, x0:x0 + DW1]
            eq1 = np.zeros((64, DW1), bool)
            xs = np.arange(x0, x0 + DW1) + dc
            ok = (xs >= 0) & (xs < W)
            eq1[:, ok] = a1[:, ok] == lbp[rows + dr][:, xs[ok]]
            shm1[64 * g:64 * g + 64, j, :] = np.where(eq1, -1.0, 1.0)
    return shm0.reshape(128, 12 * DW0), shm1.reshape(128, 12 * DW1)


def _host_inputs(logits, labels):
    in_maps = []
    for k in range(N_CORES):
        img, half = k // 2, k % 2
        g0 = half * ROWS_PER_CORE
        hi = min(H, g0 + LGROWS)
        lg = np.zeros((C, LGROWS, W), np.float32)
        lg[:, : hi - g0] = logits[img, :, g0:hi]
        lbp = np.full((LGROWS, W), -1.0, np.float32)
        lbp[: hi - g0] = labels[img, g0:hi].astype(np.float32)

        lgf = np.zeros((132, 2 + C * DW0 + 2), np.float32)
        lgf[:131, 2:2 + C * DW0] = (
            lg[:, 0:131].transpose(1, 0, 2).reshape(131, C * DW0))
        lgp1 = np.zeros((2, P1ROWS, 2 + C * DW1 + 2), np.float32)
        for g in range(2):
            x0 = GROUP_X0[g]
            lgp1[g, :, 2:2 + C * DW1] = (
                lg[:, 128:128 + P1ROWS, x0:x0 + DW1]
                .transpose(1, 0, 2).reshape(P1ROWS, C * DW1))
        shm0, shm1 = _sign_maps(lbp)
        in_maps.append({
            "lgf": lgf.astype(ml_dtypes.bfloat16),
            "lgp1": lgp1.astype(ml_dtypes.bfloat16),
            "shm0": shm0.astype(ml_dtypes.bfloat16),
            "shm1": shm1.astype(ml_dtypes.bfloat16),
            "ident": np.eye(128, dtype=np.float32).astype(ml_dtypes.bfloat16),
        })
    return in_maps


def _combine(accs_list, bcols_list):
    global _BW
    if _BW is None:
        _BW = _border_weights()
    total = 0.0
    for k in range(N_CORES):
        acc = accs_list[k].astype(np.float64)
        bc = bcols_list[k].astype(np.float64)
        g0 = (k % 2) * ROWS_PER_CORE
        for pi in range(2):
            p = np.arange(128)
            gy = g0 + p if pi == 0 else g0 + 128 + (p % 64)
            for si, (dr, dc) in enumerate(SHIFTS):
                mult = 1.0 if (dr, dc) == (0, 0) else 2.0
                wxc = float(sum(1 for ca in range(K) if 0 <= ca + dc < K))
                wy = _wy_profile(dr, gy)
                idx = pi * NS + si
                wb = _BW[:, idx * 4: idx * 4 + 4]
                full = acc[:, idx]
                border = (bc[:, idx * 4: idx * 4 + 4] * wb).sum(1)
                total += mult * np.sum(wy * (wxc * full + border))
    return total / (N * 81 * HP * WP)


_NC = None


def _build():
    global _NC
    if _NC is not None:
        return _NC
    from concourse import bacc, mybir
    import concourse.tile as tile

    f32 = mybir.dt.float32
    bf16 = mybir.dt.bfloat16
    Alu = mybir.AluOpType
    AF = mybir.ActivationFunctionType

    from concourse.hw_specs import get_activation_tables as _gat
    _keep = "natural_log_exp_and_others"
    _mine = {AF.Exp, AF.Ln, AF.Square, AF.Copy}

    def _gat_filtered(arch):
        t = _gat(arch)
        for name in t:
            if name != _keep:
                t[name] = t[name] - _mine
        return t

    bacc.get_activation_tables = _gat_filtered

    nc = bacc.Bacc("TRN2", target_bir_lowering=False, debug=False, num_devices=N_CORES)
    lgf = nc.dram_tensor("lgf", (132, 2 + C * DW0 + 2), bf16, kind="ExternalInput")
    lgp1 = nc.dram_tensor("lgp1", (2, P1ROWS, 2 + C * DW1 + 2), bf16,
                          kind="ExternalInput")
    shm0 = nc.dram_tensor("shm0", (128, 12 * DW0), bf16, kind="ExternalInput")
    shm1 = nc.dram_tensor("shm1", (128, 12 * DW1), bf16, kind="ExternalInput")
    identd = nc.dram_tensor("ident", (128, 128), bf16, kind="ExternalInput")
    accs = nc.dram_tensor("accs", (128, COLS), f32, kind="ExternalOutput")
    bcols = nc.dram_tensor("bcols", (128, BCOLS), bf16, kind="ExternalOutput")

    with tile.TileContext(nc) as tc:
        with tc.tile_pool(name="persist", bufs=1) as pool, \
             tc.tile_pool(name="work", bufs=2) as wpool, \
             tc.tile_pool(name="psum", bufs=2, space="PSUM") as psum_pool:
            accs_t = pool.tile([128, COLS], f32, name="accs_t")
            idt = pool.tile([128, 128], bf16, name="idt")
            nc.sync.dma_start(idt[:, :], identd[:, :])

            for pi, (DW, order) in enumerate(((DW0, ORDER0), (DW1, ORDER1))):
                FW = C * DW
                T = {}
                for dr in range(K):
                    t = pool.tile([128, FW + 4], bf16, tag=f"T{dr}_{pi}",
                                  name=f"T{dr}_{pi}")
                    if pi == 0:
                        nchunk = 16 if dr == 0 else 8
                        bnds = [round((FW + 4) * i / nchunk) for i in range(nchunk + 1)]
                        for ci, (c0, c1) in enumerate(zip(bnds[:-1], bnds[1:])):
                            eng = nc.gpsimd if ci % 2 == 0 else nc.sync
                            eng.dma_start(t[:, c0:c1], lgf[dr:dr + 128, c0:c1])
                    else:
                        for g in range(2):
                            nc.gpsimd.dma_start(t[64 * g:64 * g + 64, :],
                                                lgp1[g, dr:dr + 64, :])
                    T[dr] = t

                shm_t = pool.tile([128, 12 * DW], bf16, tag=f"shm_{pi}",
                                  name=f"shm_{pi}")
                shmd = shm0 if pi == 0 else shm1
                half = 6 * DW
                nc.sync.dma_start(shm_t[:, 0:half], shmd[:, 0:half])
                nc.sync.dma_start(shm_t[:, half:], shmd[:, half:])

                l1 = pool.tile([128, NS, DW], bf16, tag=f"l1_{pi}", name=f"l1_{pi}")

                convert = CONVERT0 if pi == 0 else CONVERT1
                qh = {}
                for d in sorted({0} | {dr for (dr, dc) in convert}):
                    q = pool.tile([128, DW + 4], bf16, tag=f"qh{d}_{pi}",
                                  name=f"qh{d}_{pi}")
                    nc.gpsimd.memset(q[:, 0:2], 0)
                    nc.gpsimd.memset(q[:, DW + 2:DW + 4], 0)
                    qh[d] = q

                def tree(pb, out_ap):
                    nc.vector.tensor_tensor(pb[:, 0:8 * DW], pb[:, 0:8 * DW],
                                            pb[:, 8 * DW:16 * DW], Alu.add)
                    nc.vector.tensor_tensor(pb[:, 0:4 * DW], pb[:, 0:4 * DW],
                                            pb[:, 4 * DW:8 * DW], Alu.add)
                    nc.vector.tensor_tensor(pb[:, 0:3 * DW], pb[:, 0:3 * DW],
                                            pb[:, 16 * DW:19 * DW], Alu.add)
                    nc.vector.tensor_tensor(pb[:, 0:2 * DW], pb[:, 0:2 * DW],
                                            pb[:, 2 * DW:4 * DW], Alu.add)
                    nc.vector.tensor_tensor(out_ap, pb[:, 0:DW],
                                            pb[:, DW:2 * DW], Alu.add)

                def softplus_accum(si, corr_ap, center):
                    idx = pi * NS + si
                    u = wpool.tile([128, DW], f32, tag="u", bufs=4, name=f"u_{pi}_{si}")
                    if center:
                        nc.scalar.activation(u[:, :], corr_ap, AF.Exp, scale=-1.0)
                    else:
                        j = NONCENTER.index(si)
                        wt = wpool.tile([128, DW], bf16, tag="wt", bufs=4,
                                        name=f"wt_{pi}_{si}")
                        nc.vector.tensor_tensor(wt[:, :],
                                                shm_t[:, j * DW:(j + 1) * DW],
                                                corr_ap, Alu.mult)
                        nc.scalar.activation(u[:, :], wt[:, :], AF.Exp)
                    nc.scalar.activation(
                        l1[:, si, :], u[:, :], AF.Ln, bias=1.0,
                        accum_out=accs_t[:, idx:idx + 1])

                def emit_center():
                    si = SHIFTS.index((0, 0))
                    pb = wpool.tile([128, FW], bf16, tag="pb", bufs=4, name=f"pb_{pi}_c")
                    h = (FW // 2) & ~1
                    nc.scalar.activation(pb[:, 0:h], T[0][:, 2:2 + h], AF.Square)
                    nc.scalar.activation(pb[:, h:FW], T[0][:, 2 + h:2 + FW],
                                         AF.Square)
                    corr = wpool.tile([128, DW], bf16, tag="corr", bufs=4,
                                      name=f"corr_{pi}_c")
                    tree(pb, corr[:, :])
                    nc.scalar.activation(qh[0][:, 2:2 + DW], corr[:, :],
                                         AF.Copy, scale=0.5)
                    softplus_accum(si, corr[:, :], True)

                def emit_qaux(d):
                    pb = wpool.tile([128, FW], bf16, tag="pb", bufs=4, name=f"pbq{d}_{pi}")
                    nc.scalar.activation(pb[:, :], T[d][:, 2:2 + FW], AF.Square,
                                         scale=RT_HALF)
                    tree(pb, qh[d][:, 2:2 + DW])

                def emit_q1_assembly():
                    if pi == 0:
                        nc.sync.dma_start(qh[1][0:127, 2:2 + DW],
                                          qh[0][1:128, 2:2 + DW])
                        nc.sync.dma_start(qh[1][127:128, 2:2 + DW],
                                          qh[2][126:127, 2:2 + DW])
                    else:
                        for g in range(2):
                            nc.sync.dma_start(
                                qh[1][64 * g:64 * g + 63, 2:2 + DW],
                                qh[0][64 * g + 1:64 * g + 64, 2:2 + DW])
                            nc.sync.dma_start(
                                qh[1][64 * g + 63:64 * g + 64, 2:2 + DW],
                                qh[2][64 * g + 62:64 * g + 63, 2:2 + DW])

                def emit_slot_dve(dr, dc):
                    si = SHIFTS.index((dr, dc))
                    o1 = 2 + dc
                    pb = wpool.tile([128, FW], bf16, tag="pb", bufs=4,
                                    name=f"pb_{pi}_{si}")
                    corr = wpool.tile([128, DW], bf16, tag="corr", bufs=4,
                                      name=f"corr_{pi}_{si}")
                    nc.vector.tensor_tensor(pb[:, :], T[0][:, 2:2 + FW],
                                            T[dr][:, o1:o1 + FW], Alu.mult)
                    tree(pb, corr[:, :])
                    softplus_accum(si, corr[:, :], False)

                def emit_slot_pe(dr, dc):
                    si = SHIFTS.index((dr, dc))
                    o1 = 2 + dc
                    MMW = 512
                    nch = (FW + MMW - 1) // MMW
                    pb = wpool.tile([128, FW], bf16, tag="pb", bufs=4,
                                    name=f"pbS_{pi}_{si}")
                    done = 0
                    while done < nch:
                        take = min(4, nch - done)
                        ps = psum_pool.tile([128, 2048], f32, tag="ps", bufs=2,
                                            name=f"ps_{pi}_{si}_{done}")
                        for k in range(take):
                            c0 = (done + k) * MMW
                            w = min(MMW, FW - c0)
                            nc.tensor.matmul(ps[:, k * MMW:k * MMW + w],
                                             idt[:, :], T[0][:, 2 + c0:2 + c0 + w],
                                             start=True, stop=False)
                            nc.tensor.matmul(ps[:, k * MMW:k * MMW + w],
                                             idt[:, :],
                                             T[dr][:, o1 + c0:o1 + c0 + w],
                                             start=False, stop=True)
                        w2 = min(2048, FW - done * MMW)
                        nc.scalar.activation(pb[:, done * MMW:done * MMW + w2],
                                             ps[:, 0:w2], AF.Square,
                                             scale=RT_HALF)
                        done += take
                    corr = wpool.tile([128, DW], bf16, tag="corr", bufs=4,
                                      name=f"corr_{pi}_{si}")
                    tree(pb, corr[:, :])
                    nc.vector.tensor_tensor(corr[:, :], corr[:, :],
                                            qh[0][:, 2:2 + DW], Alu.subtract)
                    nc.vector.tensor_tensor(corr[:, :], corr[:, :],
                                            qh[dr][:, o1:o1 + DW], Alu.subtract)
                    softplus_accum(si, corr[:, :], False)

                emit_center()
                for (dr, dc) in order[1:]:
                    if (dr, dc) == (1, 0):
                        emit_slot_dve(1, 0)
                        need = {d for (d2, c2) in convert for d in (d2,)} - {0}
                        if 2 in need:
                            emit_qaux(2)
                            if 1 in need:
                                emit_q1_assembly()
                        elif 1 in need:
                            emit_qaux(1)
                        continue
                    if (dr, dc) in convert:
                        emit_slot_pe(dr, dc)
                    else:
                        emit_slot_dve(dr, dc)

                bc = pool.tile([128, NS, 4], bf16, tag=f"bc_{pi}", name=f"bc_{pi}")
                nc.scalar.copy(bc[:, :, 0:2], l1[:, :, 0:2])
                nc.scalar.copy(bc[:, :, 2:4], l1[:, :, DW - 2:DW])
                nc.sync.dma_start(
                    bcols[:, pi * NS * 4:(pi + 1) * NS * 4],
                    bc[:, :, :].rearrange("p s b -> p (s b)"))
                nc.sync.dma_start(accs[:, pi * NS:(pi + 1) * NS],
                                  accs_t[:, pi * NS:(pi + 1) * NS])

    nc.finalize()
    _NC = nc
    return nc


def kernel(logits, labels):
    nc = _build()
    in_maps = _host_inputs(np.asarray(logits, np.float32), np.asarray(labels))
    from concourse.bass_utils import run_bass_kernel_spmd
    res = run_bass_kernel_spmd(nc, in_maps, core_ids=list(range(N_CORES)))
    accs_list = [res.results[k]["accs"] for k in range(N_CORES)]
    bcols_list = [res.results[k]["bcols"] for k in range(N_CORES)]
    return np.array(_combine(accs_list, bcols_list), np.float32)
